# revision 14
# baseline (speedup 1.0000x reference)
"""Trainium2 Bass kernel for BuiltSWAP: out = (state_re + i*state_im) @ M.

M is a real [8192, 8192] matrix (a 0/1 SWAP-gate permutation in practice, but
treated as a dense matrix).  Since M is real, the complex matmul decomposes
into two real matmuls sharing the same rhs:

    out_re = state_re @ M          out_im = state_im @ M

Strategy (8 NeuronCores, column-sharded tensor parallelism):
  - Shard M's columns across the 8 cores (1024 each): core c computes
    state[64, 8192] @ M[:, c*1024:(c+1)*1024] for re and im; outputs are
    concatenated on the host.
  - Fuse re and im into a single 128-col stationary operand (64 re + 64 im
    rows of state^T) so the 128x128 PE array is fully used.
  - Precision: split fp32 state into fp16 hi + fp16 lo (lo pre-scaled by 2^22
    so it stays in fp16 normal range): state = hi + lo/2^22 carries ~22
    significand bits.  Two matmul passes accumulate into separate fp32 PSUM
    banks; a DVE combine computes hi_psum + lo_psum * 2^-22.
  - M's 0/1 entries are exact in fp8e4m3, so M is stored/streamed as fp8
    (mixed-dtype matmul: fp16 stationary x fp8 moving), halving its HBM
    traffic vs bf16.
  Measured: rel err ~1.2e-7 per element vs fp64 reference; ~55-60us steady
  state per kernel on 8 cores (PE-streaming bound).
"""

import numpy as np
import ml_dtypes

BATCH = 64
N = 8192
NCORES = 8
COLS = N // NCORES          # 1024 output columns per core
P = 128                     # partitions
KT = N // P                 # 64 k-tiles
NCH = COLS // 512           # 2 psum chunks of 512
KBLK = 8                    # k-tiles per M DMA block
NBLK = KT // KBLK           # 8 DMA blocks

f8e4 = ml_dtypes.float8_e4m3
SCALE_BITS = 22
SCALE = float(2 ** SCALE_BITS)
INV_SCALE = float(2.0 ** (-SCALE_BITS))

_cached = {}


def _fp8_exact(M):
    # cheap exactness check: fp8e4m3 round-trips M losslessly?
    sample = M[:: 64, :: 64]
    if not np.array_equal(sample.astype(f8e4).astype(np.float32), sample):
        return False
    return np.array_equal(M.astype(f8e4).astype(np.float32), M)


def _build_program(reps=1, serialize=False, m_dt="fp8"):
    # reps>1 repeats the whole pipeline inside one NEFF (for benchmarking);
    # serialize adds an all-engine barrier between reps so the per-rep slope
    # approximates a single-shot kernel execution.
    import concourse.mybir as mybir
    import concourse.tile as tile
    from concourse import bacc

    mdt = {"fp8": mybir.dt.float8e4, "bf16": mybir.dt.bfloat16}[m_dt]
    nc = bacc.Bacc("TRN2", target_bir_lowering=False, debug=False)
    st_d = nc.declare_dram_parameter("st", [P, KT, 256], mybir.dt.float16, isOutput=False)
    m_d = nc.declare_dram_parameter("m", [P, KT, NCH, 512], mdt, isOutput=False)
    out_d = nc.declare_dram_parameter("out", [P, COLS], mybir.dt.float32, isOutput=True)

    with tile.TileContext(nc) as tc:
        with (
            tc.tile_pool(name="stp", bufs=1) as stp,
            tc.tile_pool(name="mp", bufs=3) as mp,
            tc.tile_pool(name="op", bufs=1) as op,
            tc.tile_pool(name="ps", bufs=1, space="PSUM") as ps,
        ):
            st_sb = stp.tile([P, KT, 256], mybir.dt.float16)
            # split the state load so the first matmuls aren't gated on 4MB
            for kb in range(NBLK):
                nc.sync.dma_start(
                    st_sb[:, kb * KBLK:(kb + 1) * KBLK, :],
                    st_d[:, kb * KBLK:(kb + 1) * KBLK, :],
                )
            for _rep in range(reps):
                if serialize and reps > 1:
                    tc.strict_bb_all_engine_barrier()
                out_sb = op.tile([P, COLS], mybir.dt.float32, name="out_sb")
                ps_hi = [
                    ps.tile([P, 512], mybir.dt.float32, name=f"ps_hi{i}")
                    for i in range(NCH)
                ]
                ps_lo = [
                    ps.tile([P, 512], mybir.dt.float32, name=f"ps_lo{i}")
                    for i in range(NCH)
                ]
                for kb in range(NBLK):
                    m_sb = mp.tile([P, KBLK, NCH, 512], mdt, name="m_sb")
                    nc.sync.dma_start(m_sb[:], m_d[:, kb * KBLK:(kb + 1) * KBLK, :, :])
                    for kj in range(KBLK):
                        ko = kb * KBLK + kj
                        # pass-major order: the stationary operand (hi or lo
                        # state tile) is reused across both n-chunks, halving
                        # LDWEIGHTS traffic vs alternating hi/lo per chunk
                        for pss, c0 in ((ps_hi, 0), (ps_lo, 128)):
                            for nch in range(NCH):
                                nc.tensor.matmul(
                                    pss[nch][:],
                                    st_sb[:, ko, c0:c0 + 128],
                                    m_sb[:, kj, nch, :],
                                    start=(ko == 0),
                                    stop=(ko == KT - 1),
                                )
                for nch in range(NCH):
                    sl = slice(nch * 512, (nch + 1) * 512)
                    nc.vector.tensor_scalar_mul(out_sb[:, sl], ps_lo[nch][:], INV_SCALE)
                    nc.vector.tensor_add(out_sb[:, sl], out_sb[:, sl], ps_hi[nch][:])
                nc.sync.dma_start(out_d[:], out_sb[:])
    nc.compile()
    return nc


def _get_program(m_dt="fp8"):
    key = f"nc_{m_dt}"
    if key not in _cached:
        _cached[key] = _build_program(m_dt=m_dt)
    return _cached[key]


def _prep_inputs(state_re, state_im, M, m_dt="fp8"):
    # Stationary layout: [8192, 256] fp16 where cols 0:64 re_hi, 64:128 im_hi,
    # 128:192 re_lo*2^22, 192:256 im_lo*2^22; tiled to [128 part, 64 kt, 256].
    S = np.empty((N, P), dtype=np.float32)
    S[:, :BATCH] = state_re.T
    S[:, BATCH:] = state_im.T
    hi = S.astype(np.float16)
    lo = ((S - hi.astype(np.float32)) * SCALE).astype(np.float16)
    stall = np.concatenate([hi, lo], axis=1)  # [8192, 256] fp16
    st_tiled = np.ascontiguousarray(
        stall.reshape(KT, P, 256).transpose(1, 0, 2)
    )  # [128, 64, 256]

    Mb = M.astype(f8e4 if m_dt == "fp8" else ml_dtypes.bfloat16)
    m_tiles = []
    for c in range(NCORES):
        shard = Mb[:, c * COLS:(c + 1) * COLS]
        m_tiles.append(
            np.ascontiguousarray(
                shard.reshape(KT, P, NCH, 512).transpose(1, 0, 2, 3)
            )
        )  # [128, 64, 2, 512]
    return st_tiled, m_tiles


def run_on_hw(state_re, state_im, M, trace=False):
    from concourse.bass_utils import run_bass_kernel_spmd

    state_re = np.asarray(state_re, dtype=np.float32)
    state_im = np.asarray(state_im, dtype=np.float32)
    M = np.asarray(M, dtype=np.float32)
    # fp8e4m3 storage of M is exact only for values with <=4 significand
    # bits; the BuiltSWAP gate matrix is 0/1 so the fast path always takes
    # fp8.  Fall back to bf16 if an unexpected M shows up.
    m_dt = "fp8" if _fp8_exact(M) else "bf16"
    nc = _get_program(m_dt)
    st_tiled, m_tiles = _prep_inputs(state_re, state_im, M, m_dt)
    in_maps = [{"st": st_tiled, "m": m_tiles[c]} for c in range(NCORES)]
    res = run_bass_kernel_spmd(
        nc, in_maps, list(range(NCORES)), trace=trace,
        trace_cores=list(range(NCORES)) if trace else None,
    )
    full = np.concatenate([res.results[c]["out"] for c in range(NCORES)], axis=1)
    out = (full[:BATCH] + 1j * full[BATCH:]).astype(np.complex64)
    return out, res


def kernel(state_re, state_im, M):
    out, _ = run_on_hw(state_re, state_im, M, trace=False)
    return out


# revision 15
# speedup vs baseline: 1.5476x; 1.5476x over previous
"""Trainium2 Bass kernel for BuiltSWAP: out = (state_re + i*state_im) @ M.

M is a real [8192, 8192] matrix (a 0/1 SWAP-gate permutation in practice, but
treated as a dense matrix).  Since M is real, the complex matmul decomposes
into two real matmuls sharing the same rhs:

    out_re = state_re @ M          out_im = state_im @ M

Strategy (8 NeuronCores, column-sharded tensor parallelism):
  - Shard M's columns across the 8 cores (1024 each): core c computes
    state[64, 8192] @ M[:, c*1024:(c+1)*1024] for re and im; outputs are
    concatenated on the host.
  - Fuse re and im into a single 128-col stationary operand (64 re + 64 im
    rows of state^T) so the 128x128 PE array is fully used.
  - Precision: split fp32 state into fp16 hi + fp16 lo (lo pre-scaled by 2^22
    so it stays in fp16 normal range): state = hi + lo/2^22 carries ~22
    significand bits.  Two matmul passes accumulate into separate fp32 PSUM
    banks; a DVE combine computes hi_psum + lo_psum * 2^-22.
  - M's 0/1 entries are exact in fp8e4m3, so M is stored/streamed as fp8
    (mixed-dtype matmul: fp16 stationary x fp8 moving), halving its HBM
    traffic vs bf16.
  Measured: rel err ~1.2e-7 per element vs fp64 reference; ~55-60us steady
  state per kernel on 8 cores (PE-streaming bound).
"""

import numpy as np
import ml_dtypes

BATCH = 64
N = 8192
NCORES = 8
COLS = N // NCORES          # 1024 output columns per core
P = 128                     # partitions
KT = N // P                 # 64 k-tiles
NCH = COLS // 512           # 2 psum chunks of 512
KBLK = 8                    # k-tiles per M DMA block
NBLK = KT // KBLK           # 8 DMA blocks

f8e4 = ml_dtypes.float8_e4m3
SCALE_BITS = 22
SCALE = float(2 ** SCALE_BITS)
INV_SCALE = float(2.0 ** (-SCALE_BITS))

_cached = {}


def _fp8_exact(M):
    # cheap exactness check: fp8e4m3 round-trips M losslessly?
    sample = M[:: 64, :: 64]
    if not np.array_equal(sample.astype(f8e4).astype(np.float32), sample):
        return False
    return np.array_equal(M.astype(f8e4).astype(np.float32), M)


def _build_program(reps=1, serialize=False, m_dt="fp8"):
    # reps>1 repeats the whole pipeline inside one NEFF (for benchmarking);
    # serialize adds an all-engine barrier between reps so the per-rep slope
    # approximates a single-shot kernel execution.
    import concourse.mybir as mybir
    import concourse.tile as tile
    from concourse import bacc

    mdt = {"fp8": mybir.dt.float8e4, "bf16": mybir.dt.bfloat16}[m_dt]
    nc = bacc.Bacc("TRN2", target_bir_lowering=False, debug=False)
    st_d = nc.declare_dram_parameter("st", [P, KT, 256], mybir.dt.float16, isOutput=False)
    m_d = nc.declare_dram_parameter("m", [P, KT, NCH, 512], mdt, isOutput=False)
    out_d = nc.declare_dram_parameter("out", [P, COLS], mybir.dt.float32, isOutput=True)

    with tile.TileContext(nc) as tc:
        with (
            tc.tile_pool(name="stp", bufs=1) as stp,
            tc.tile_pool(name="mp", bufs=3) as mp,
            tc.tile_pool(name="op", bufs=1) as op,
            tc.tile_pool(name="ps", bufs=1, space="PSUM") as ps,
        ):
            st_sb = stp.tile([P, KT, 256], mybir.dt.float16)
            # split the state load so the first matmuls aren't gated on 4MB
            for kb in range(NBLK):
                nc.sync.dma_start(
                    st_sb[:, kb * KBLK:(kb + 1) * KBLK, :],
                    st_d[:, kb * KBLK:(kb + 1) * KBLK, :],
                )
            # dummy matmuls on a zeroed scratch tile run during the initial
            # DMA wait and release the PE HAM clock throttle (1.2 -> 2.4 GHz)
            # before the real matmuls start (measured ~5us single-shot win)
            wsb = stp.tile([P, 128], mybir.dt.float16, name="wsb")
            nc.vector.memset(wsb[:], 0.0)
            wps = ps.tile([P, 128], mybir.dt.float32, name="wps")
            for _rep in range(reps):
                if serialize and reps > 1:
                    tc.strict_bb_all_engine_barrier()
                for _ in range(40):
                    nc.tensor.matmul(wps[:], wsb[:], wsb[:], start=True, stop=True)
                out_sb = op.tile([P, COLS], mybir.dt.float32, name="out_sb")
                ps_hi = [
                    ps.tile([P, 512], mybir.dt.float32, name=f"ps_hi{i}")
                    for i in range(NCH)
                ]
                ps_lo = [
                    ps.tile([P, 512], mybir.dt.float32, name=f"ps_lo{i}")
                    for i in range(NCH)
                ]
                for kb in range(NBLK):
                    m_sb = mp.tile([P, KBLK, NCH, 512], mdt, name="m_sb")
                    nc.sync.dma_start(m_sb[:], m_d[:, kb * KBLK:(kb + 1) * KBLK, :, :])
                    for kj in range(KBLK):
                        ko = kb * KBLK + kj
                        # pass-major order: the stationary operand (hi or lo
                        # state tile) is reused across both n-chunks, halving
                        # LDWEIGHTS traffic vs alternating hi/lo per chunk
                        for pss, c0 in ((ps_hi, 0), (ps_lo, 128)):
                            for nch in range(NCH):
                                nc.tensor.matmul(
                                    pss[nch][:],
                                    st_sb[:, ko, c0:c0 + 128],
                                    m_sb[:, kj, nch, :],
                                    start=(ko == 0),
                                    stop=(ko == KT - 1),
                                )
                for nch in range(NCH):
                    sl = slice(nch * 512, (nch + 1) * 512)
                    nc.vector.tensor_scalar_mul(out_sb[:, sl], ps_lo[nch][:], INV_SCALE)
                    nc.vector.tensor_add(out_sb[:, sl], out_sb[:, sl], ps_hi[nch][:])
                nc.sync.dma_start(out_d[:], out_sb[:])
    nc.compile()
    return nc


def _get_program(m_dt="fp8"):
    key = f"nc_{m_dt}"
    if key not in _cached:
        _cached[key] = _build_program(m_dt=m_dt)
    return _cached[key]


def _prep_inputs(state_re, state_im, M, m_dt="fp8"):
    # Stationary layout: [8192, 256] fp16 where cols 0:64 re_hi, 64:128 im_hi,
    # 128:192 re_lo*2^22, 192:256 im_lo*2^22; tiled to [128 part, 64 kt, 256].
    S = np.empty((N, P), dtype=np.float32)
    S[:, :BATCH] = state_re.T
    S[:, BATCH:] = state_im.T
    hi = S.astype(np.float16)
    lo = ((S - hi.astype(np.float32)) * SCALE).astype(np.float16)
    stall = np.concatenate([hi, lo], axis=1)  # [8192, 256] fp16
    st_tiled = np.ascontiguousarray(
        stall.reshape(KT, P, 256).transpose(1, 0, 2)
    )  # [128, 64, 256]

    Mb = M.astype(f8e4 if m_dt == "fp8" else ml_dtypes.bfloat16)
    m_tiles = []
    for c in range(NCORES):
        shard = Mb[:, c * COLS:(c + 1) * COLS]
        m_tiles.append(
            np.ascontiguousarray(
                shard.reshape(KT, P, NCH, 512).transpose(1, 0, 2, 3)
            )
        )  # [128, 64, 2, 512]
    return st_tiled, m_tiles


def run_on_hw(state_re, state_im, M, trace=False):
    from concourse.bass_utils import run_bass_kernel_spmd

    state_re = np.asarray(state_re, dtype=np.float32)
    state_im = np.asarray(state_im, dtype=np.float32)
    M = np.asarray(M, dtype=np.float32)
    # fp8e4m3 storage of M is exact only for values with <=4 significand
    # bits; the BuiltSWAP gate matrix is 0/1 so the fast path always takes
    # fp8.  Fall back to bf16 if an unexpected M shows up.
    m_dt = "fp8" if _fp8_exact(M) else "bf16"
    nc = _get_program(m_dt)
    st_tiled, m_tiles = _prep_inputs(state_re, state_im, M, m_dt)
    in_maps = [{"st": st_tiled, "m": m_tiles[c]} for c in range(NCORES)]
    res = run_bass_kernel_spmd(
        nc, in_maps, list(range(NCORES)), trace=trace,
        trace_cores=list(range(NCORES)) if trace else None,
    )
    full = np.concatenate([res.results[c]["out"] for c in range(NCORES)], axis=1)
    out = (full[:BATCH] + 1j * full[BATCH:]).astype(np.complex64)
    return out, res


def kernel(state_re, state_im, M):
    out, _ = run_on_hw(state_re, state_im, M, trace=False)
    return out
